# revision 46
# baseline (speedup 1.0000x reference)
"""AttrPredLoss_40 focal-BCE loss kernel for Trainium2 (8 NeuronCores, data parallel).

Math (per sample row, 18 selected attrs j):
    p   = pred[:, SEL]                      # in (0,1)
    t   = label in {0,1}
    d   = t - p
    # BCE log arg: t==1 -> p, t==0 -> 1-p  ==  1-|d|
    l   = max(ln(1-|d|), -100)              # sum_j l = -18*bce_mean
    # focal weight: (t?0.8:0.2) * (t?1-p:p)^2 == (t+1/3)*0.6*d^2
    f   = (t + 1/3) * 0.6 * d^2
    loss = sum_rows (sum_j f) * (-1/18) * (sum_j l)

The double reduction + row product is computed on the TensorEngine:
    total = sum_r (sum_j f_rj)(sum_k l_rk) = sum over diagonal 18x18 blocks of
    G += f_chunk^T @ l_chunk  accumulated in PSUM over all 128-row groups,
    chunked 4 row-groups (72 cols) per matmul.
"""

import math
from contextlib import ExitStack

import numpy as np

import concourse.bacc as bacc
import concourse.bass as bass
import concourse.mybir as mybir
import concourse.tile as tile
from concourse.bass_utils import run_bass_kernel_spmd

F32 = mybir.dt.float32
I32 = mybir.dt.int32
BF16 = mybir.dt.bfloat16
F16 = mybir.dt.float16
ALU = mybir.AluOpType
ACTF = mybir.ActivationFunctionType

# selected attribute indices, as contiguous runs: (label_col, pred_col, len)
RUNS = [
    (0, 4, 1),
    (1, 8, 2),
    (3, 11, 1),
    (4, 13, 6),
    (10, 20, 1),
    (11, 22, 2),
    (13, 26, 1),
    (14, 29, 3),
    (17, 36, 1),
]
# same 18 columns as 6 ops: (label_col0, pred_col0, nrun, lstep, pstep, len)
# pairs of runs whose spacing is arithmetic on BOTH the label and pred side
MERGED_RUNS = [
    (0, 4, 1, 0, 0, 1),     # {4}
    (1, 8, 2, 10, 14, 2),   # {8,9} + {22,23}
    (3, 11, 2, 10, 15, 1),  # {11} + {26}
    (4, 13, 1, 0, 0, 6),    # {13..18}
    (10, 20, 2, 7, 16, 1),  # {20} + {36}
    (14, 29, 1, 0, 0, 3),   # {29,30,31}
]
NSEL = 18
NCOL = 40

N_CORES = 8
B = 1_048_576
B_SHARD = B // N_CORES  # 131072
P = 128  # SBUF partitions
U = 4  # row-groups per matmul chunk (must divide r; 18*4 = 72 partitions)
GDIM = NSEL * U  # 72


def build_nc(
    b_shard: int = B_SHARD,
    r: int = 128,
    label_cast: str = "dma",
    loop_n: int = 1,
    io_bufs: int = 2,
    variant: str = "full",
    tile_sizes: list | None = None,
    label_bufs: int | None = None,
    clamp_engine: str = "dve",
    loop_staggered: bool = False,
    sw_pipe: bool = False,
    relu_clamp: bool = False,
    eps_ln: bool = False,
    act_order: str = "sq_first",
    mid_bufs: int = 2,
    abs_engine: str = "act",
    mid16: bool = False,
    pred16: bool = False,
    sub_engine: str = "dve",
    pack: bool = False,
    d16: bool = False,
    n_gbank: int = 1,
    label_queue: str = "sync",
    passes: int = 1,
    back_first: bool = False,
    dma_span: int = 1,
    s6_psum: bool = False,
    big_mode: bool = False,
    eps_imm: bool = False,
    sw_depth: int = 1,
):
    """Build the per-core Bass module. b_shard rows per core; r rows per
    partition per tile (or explicit tile_sizes list summing to nrows).
    loop_n>1 wraps the body in a device-side For loop (same data re-read
    each iteration) for wall-clock benchmarking."""
    assert b_shard % P == 0
    nrows = b_shard // P
    if tile_sizes is None:
        assert nrows % r == 0
        tile_sizes = [r] * (nrows // r)
    assert sum(tile_sizes) == nrows, (tile_sizes, nrows)
    assert all(t % U == 0 for t in tile_sizes)

    nc = bacc.Bacc("TRN2", target_bir_lowering=False, debug=False)

    pred = nc.dram_tensor("pred", [b_shard, NCOL], F32, kind="ExternalInput")
    label = nc.dram_tensor("label", [b_shard, NSEL], I32, kind="ExternalInput")
    assert n_gbank == 1 or not relu_clamp
    gw = GDIM + 1 if relu_clamp else n_gbank * GDIM
    gout = nc.dram_tensor("g_out", [GDIM, gw], F32, kind="ExternalOutput")

    # blocked layout: partition p holds rows [p*nrows, (p+1)*nrows)
    pred_r = pred.ap().rearrange("(p n) c -> p n c", p=P)
    label_r = label.ap().rearrange("(p n) c -> p n c", p=P)

    s6scale = math.sqrt(0.6)

    with tile.TileContext(nc) as tc, ExitStack() as ctx:
        io = ctx.enter_context(tc.tile_pool(name="io", bufs=io_bufs))
        psum2 = None
        if s6_psum:
            psum2 = ctx.enter_context(
                tc.tile_pool(name="psum2", bufs=mid_bufs, space="PSUM")
            )
        da = None
        if big_mode:
            # d and a share one 2-buffer pool (both die within their wave:
            # d after the sign-clear, a after Square+Ln read it)
            da = ctx.enter_context(tc.tile_pool(name="da", bufs=2))
        if label_bufs is None:
            iol = io
        else:
            iol = ctx.enter_context(tc.tile_pool(name="iol", bufs=label_bufs))
        mid = ctx.enter_context(tc.tile_pool(name="mid", bufs=mid_bufs))
        singles = ctx.enter_context(tc.tile_pool(name="singles", bufs=1))
        psum = ctx.enter_context(tc.tile_pool(name="psum", bufs=1, space="PSUM"))

        Gb = []
        for b_i in range(n_gbank):
            # pad each accumulator to a full 2KiB PSUM bank: matmul
            # start=True clears has_written bits bank-wide, so two
            # accumulation chains sharing a bank would corrupt each other
            Gb.append(
                psum.tile(
                    [GDIM, GDIM], F32, name=f"Gbank{b_i}", tag=f"G{b_i}",
                    padded_shape=[GDIM, 512],
                )
            )
        G = Gb[0]
        G2 = None
        ones1 = None
        if relu_clamp:
            G2 = psum.tile([GDIM, 1], F32)
            ones1 = singles.tile([P, 1], F16)
            nc.vector.memset(ones1[:], 1.0)
            c100 = singles.tile([P, 1], F32)
            nc.vector.memset(c100[:], 100.0)
        ceps = None
        if eps_ln:
            ceps = singles.tile([P, 1], F32)
            nc.vector.memset(ceps[:], 1.0 + 2.0**-23)

        total_chunks = sum(t // U for t in tile_sizes) * passes
        assert total_chunks % n_gbank == 0
        chunk_idx = 0

        if loop_n > 1:
            loop_cm = tc.For_i(0, loop_n, 1, staggered_reset=loop_staggered)
        else:
            loop_cm = None
        if loop_cm is not None:
            ctx.enter_context(loop_cm)

        def emit_front(rt, rsl, pt=None, tt=None):
            """DMA + ops that depend only on this tile's DMA. When pt/tt are
            given (dma_span>1), the DMA was already issued for a span of
            tiles and pt/tt are sub-views of the span buffer."""
            if pt is not None:
                return emit_compute(rt, pt, tt)
            if pred16:
                # SWDGE cast f32 -> bf16 during the DMA (halves pred SBUF)
                pt = io.tile([P, rt, NCOL], BF16, tag="pred")
                nc.gpsimd.dma_start(out=pt[:], in_=pred_r[:, rsl, :])
            else:
                pt = io.tile([P, rt, NCOL], F32, tag="pred")
                nc.sync.dma_start(out=pt[:], in_=pred_r[:, rsl, :])

            if label_cast == "dma":
                tt = iol.tile([P, rt, NSEL], F32, tag="label")
                nc.gpsimd.dma_start(out=tt[:], in_=label_r[:, rsl, :])
            elif label_cast == "dma16":
                # SWDGE cast i32 -> bf16 during the DMA; 16-bit labels let
                # the f STT run in 2x_1P mode when s6 is also bf16
                tt = iol.tile([P, rt, NSEL], BF16, tag="label")
                nc.gpsimd.dma_start(out=tt[:], in_=label_r[:, rsl, :])
            elif label_cast == "none":
                # no cast: DVE ops read the i32 labels with on-read convert
                tt = iol.tile([P, rt, NSEL], I32, tag="label")
                lq = {"sync": nc.sync, "scalar": nc.scalar,
                      "gpsimd": nc.gpsimd}[label_queue]
                lq.dma_start(out=tt[:], in_=label_r[:, rsl, :])
            else:
                ti = iol.tile([P, rt, NSEL], I32, tag="label_i")
                nc.sync.dma_start(out=ti[:], in_=label_r[:, rsl, :])
                tt = iol.tile([P, rt, NSEL], F32, tag="label")
                nc.gpsimd.tensor_copy(
                    out=tt.rearrange("p n c -> p (n c)"),
                    in_=ti.rearrange("p n c -> p (n c)"),
                )

            if variant == "dma_only":
                return None
            return emit_compute(rt, pt, tt)

        def emit_compute(rt, pt, tt):
            dt_d = BF16 if (pred16 or pack or d16) else F32
            # d = t - p on the 18 selected columns
            if big_mode:
                d = da.tile([P, rt, NSEL], F32, tag="da")
            else:
                d = mid.tile([P, rt, NSEL], dt_d, tag="d")

            def run_view(tile3, col0, nrun, step, ln_):
                base = tile3[:]
                ap = [base.ap[0], base.ap[1]]
                if nrun > 1:
                    ap.append([step, nrun])
                ap.append([1, ln_])
                return bass.AP(tensor=base.tensor, offset=base.offset + col0, ap=ap)

            if pack:
                # gather+cast pred's 18 selected cols into a contiguous bf16
                # tile on the (otherwise idle) GPSIMD engine, then one
                # contiguous DVE sub in 2x mode
                pp = mid.tile([P, rt, NSEL], BF16, tag="pp")
                for lc0, pc0, nrun, lstep, pstep, ln_ in MERGED_RUNS:
                    nc.gpsimd.tensor_copy(
                        out=run_view(pp, lc0, nrun, lstep, ln_),
                        in_=run_view(pt, pc0, nrun, pstep, ln_),
                    )
                nc.vector.tensor_sub(
                    d.rearrange("p n c -> p (n c)"),
                    tt.rearrange("p n c -> p (n c)"),
                    pp.rearrange("p n c -> p (n c)"),
                )
            else:
                # 6 merged-run sub ops, optionally offloading the biggest
                # runs to GPSIMD
                for i, (lc0, pc0, nrun, lstep, pstep, ln_) in enumerate(MERGED_RUNS):
                    eng = nc.vector
                    if sub_engine == "gp6" or (
                        sub_engine == "gp3" and ln_ >= 2
                    ) or (sub_engine == "gp1" and ln_ == 6):
                        eng = nc.gpsimd
                    eng.tensor_sub(
                        run_view(d, lc0, nrun, lstep, ln_),
                        run_view(tt, lc0, nrun, lstep, ln_),
                        run_view(pt, pc0, nrun, pstep, ln_),
                    )

            if variant == "dpass":
                return None

            dflat = d.rearrange("p n c -> p (n c)")

            if s6_psum:
                # ScalarE writes PSUM faster than SBUF, and this takes the
                # s6 traffic off the SBUF ports entirely (bf16 to fit 2 bufs)
                s6 = psum2.tile([P, rt * NSEL], BF16, tag="s6")
            else:
                s6 = mid.tile([P, rt * NSEL], BF16 if mid16 else F32, tag="s6")
            if big_mode:
                # a = |d| via sign-clear, then BOTH Square and Ln read a
                # (d^2 == |d|^2), so d's buffer frees right after the AND
                # and d/a can share one double-buffered pool
                a3 = da.tile([P, rt, NSEL], F32, tag="da")
                a = a3.rearrange("p n c -> p (n c)")
                nc.vector.tensor_scalar(
                    a.bitcast(I32), dflat.bitcast(I32), 0x7FFFFFFF,
                    None, ALU.bitwise_and,
                )
                nc.scalar.activation(s6[:], a, ACTF.Square, scale=s6scale)
            else:
                a = mid.tile([P, rt * NSEL], dt_d, tag="a")
            if big_mode:
                pass
            elif abs_engine == "dve_and":
                # a = |d| on the DVE: clear the sign bit via a single-src
                # tensor_scalar bitwise_and on an int bitcast view (2x/4x
                # mode). Takes Abs off the ScalarE chain (ACT: Square + Ln).
                if dt_d == BF16:
                    nc.vector.tensor_scalar(
                        a[:].bitcast(mybir.dt.int16),
                        dflat.bitcast(mybir.dt.int16), 0x7FFF, None,
                        ALU.bitwise_and,
                    )
                else:
                    nc.vector.tensor_scalar(
                        a[:].bitcast(I32), dflat.bitcast(I32), 0x7FFFFFFF,
                        None, ALU.bitwise_and,
                    )
                if act_order == "sq_first":
                    nc.scalar.activation(s6[:], dflat, ACTF.Square, scale=s6scale)
            elif abs_engine == "dve_ttabs":
                # a = |d| via tensor_tensor abs_max(d, d) (1x fp32)
                nc.vector.tensor_tensor(a[:], dflat, dflat, ALU.abs_max)
                if act_order == "sq_first":
                    nc.scalar.activation(s6[:], dflat, ACTF.Square, scale=s6scale)
            elif act_order == "sq_first":
                # s6 = 0.6 * d^2   (ACT, scale applied before Square)
                nc.scalar.activation(s6[:], dflat, ACTF.Square, scale=s6scale)
                # a = |d|   (ScalarE Abs)
                nc.scalar.activation(a[:], dflat, ACTF.Abs)
            else:
                # Abs/Ln first: lc reaches the PE sooner; Square's consumer
                # (the f STT) runs one wave later anyway under sw_pipe
                nc.scalar.activation(a[:], dflat, ACTF.Abs)

            if eps_ln:
                # l = Ln(1 + 2^-23 - |d|): the epsilon keeps the argument
                # positive (p=0 -> l ~ -15.9 instead of -inf), making the
                # -100 clamp unnecessary (error impact ~3e-6 of the total).
                # Write bf16 directly for the matmul.
                l = mid.tile([P, rt * NSEL], BF16, tag="l")
                nc.scalar.activation(
                    l[:], a[:], ACTF.Ln,
                    bias=(1.0 + 2.0**-23) if eps_imm else ceps[:, 0:1],
                    scale=-1.0,
                )
            else:
                # l = Ln(1 - |d|)
                l = mid.tile([P, rt * NSEL], F32, tag="l")
                nc.scalar.activation(l[:], a[:], ACTF.Ln, bias=1.0, scale=-1.0)

            if act_order != "sq_first":
                nc.scalar.activation(s6[:], dflat, ACTF.Square, scale=s6scale)

            if variant == "acts":
                return None

            return (rt, tt, s6, l)

        def emit_back(state):
            """DVE/PE ops that consume ACT results (run one tile later when
            sw_pipe so the in-order DVE queue never waits on same-tile ACT)."""
            nonlocal chunk_idx
            rt, tt, s6, l = state
            ttflat = tt.rearrange("p n c -> p (n c)")

            # f = (t + 1/3) * s6 = (0.2 + 0.6t) * d^2  (dtype matches lc)
            f = mid.tile([P, rt * NSEL], F16 if relu_clamp else BF16, tag="f")
            nc.vector.scalar_tensor_tensor(
                f[:], ttflat, 1.0 / 3.0, s6[:], ALU.add, ALU.mult
            )

            if eps_ln:
                lc = l  # already clamped-by-construction, bf16
            elif relu_clamp:
                # lc' = relu(l + 100) = max(l,-100) + 100 on ScalarE (fp16);
                # the +100 is removed exactly via G2 = column sums of f
                lc = mid.tile([P, rt * NSEL], F16, tag="lc")
                nc.scalar.activation(lc[:], l[:], ACTF.Relu, bias=c100[:, 0:1], scale=1.0)
            else:
                # lc = max(l, -100)  -> bf16 for the matmul
                lc = mid.tile([P, rt * NSEL], BF16, tag="lc")
                if clamp_engine == "gpsimd":
                    nc.gpsimd.tensor_scalar_max(lc[:], l[:], -100.0)
                else:
                    nc.vector.tensor_scalar(lc[:], l[:], -100.0, None, ALU.max)

            if variant == "no_pe":
                return

            # G += f_chunk^T @ lc_chunk over chunks of U row-groups;
            # n_gbank>1 ping-pongs PSUM accumulators so consecutive matmuls
            # don't serialize on the same-PSUM-address accumulate
            done = 0
            while done < rt:
                u = min(U, rt - done)
                m = NSEL * u
                sl = slice(done * NSEL, (done + u) * NSEL)
                nc.tensor.matmul(
                    out=Gb[chunk_idx % n_gbank][0:m, 0:m],
                    lhsT=f[:, sl],
                    rhs=lc[:, sl],
                    start=(chunk_idx < n_gbank),
                    stop=(chunk_idx >= total_chunks - n_gbank),
                )
                if relu_clamp:
                    nc.tensor.matmul(
                        out=G2[0:m, 0:1],
                        lhsT=f[:, sl],
                        rhs=ones1[:, 0:1],
                        start=(chunk_idx == 0),
                        stop=(chunk_idx == total_chunks - 1),
                    )
                chunk_idx += 1
                done += u

        if dma_span > 1:
            assert label_cast == "none" and not pred16

        pendings = []
        for _pass in range(passes):
            row0 = 0
            idx = 0
            while idx < len(tile_sizes):
                span = 1
                while (
                    span < dma_span
                    and idx + span < len(tile_sizes)
                    and tile_sizes[idx + span] == tile_sizes[idx]
                ):
                    span += 1
                group = tile_sizes[idx : idx + span]
                idx += span
                gt = sum(group)
                ptb = ttb = None
                if span > 1:
                    # one DMA covering the whole span (fewer, larger DMAs)
                    ptb = io.tile([P, gt, NCOL], F32, tag="pred")
                    nc.sync.dma_start(
                        out=ptb[:], in_=pred_r[:, row0 : row0 + gt, :]
                    )
                    ttb = iol.tile([P, gt, NSEL], I32, tag="label")
                    lq = {"sync": nc.sync, "scalar": nc.scalar,
                          "gpsimd": nc.gpsimd}[label_queue]
                    lq.dma_start(
                        out=ttb[:], in_=label_r[:, row0 : row0 + gt, :]
                    )
                for k, rt in enumerate(group):
                    rsl = slice(row0, row0 + rt)
                    row0 += rt
                    if sw_pipe and back_first and pendings:
                        # emit the previous tile's DVE/PE consumers BEFORE
                        # this tile's front so the in-order DVE queue can run
                        # them while this tile's DMA is still in flight
                        for st_ in pendings:
                            emit_back(st_)
                        pendings = []
                    if span > 1:
                        if variant == "dma_only":
                            st = None
                        else:
                            st = emit_front(
                                rt, rsl,
                                pt=ptb[:, k * rt : (k + 1) * rt, :],
                                tt=ttb[:, k * rt : (k + 1) * rt, :],
                            )
                    else:
                        st = emit_front(rt, rsl)
                    if st is None:
                        continue
                    if not sw_pipe:
                        emit_back(st)
                    else:
                        pendings.append(st)
                        while len(pendings) > sw_depth:
                            emit_back(pendings.pop(0))
        for st_ in pendings:
            emit_back(st_)

        # epilogue: ship G (and the f column sums) to the host
        Gs = singles.tile([GDIM, gw], F32)
        if variant == "full":
            for b_i in range(n_gbank):
                nc.scalar.copy(Gs[:, b_i * GDIM : (b_i + 1) * GDIM], Gb[b_i][:])
            if relu_clamp:
                nc.scalar.copy(Gs[:, GDIM : GDIM + 1], G2[:])
        else:
            nc.vector.memset(Gs[:], 0.0)
        nc.sync.dma_start(out=gout.ap(), in_=Gs[:])

    nc.compile()
    return nc


_NC = None


TAPER = [128] * 7 + [64, 48, 16]  # smaller final tiles shorten the compute tail

# HWDGE label loads (mixed-dtype i32 reads on DVE) + TWO-tile software
# pipeline (consumers run two waves late, fully decoupling the STT/matmul
# back-phase from the ACT chain; needs deeper mid/label pools) +
# epsilon-biased Ln that makes the -100 clamp unnecessary + two
# ping-ponged PSUM accumulators so consecutive matmuls don't serialize
# on the same-bank accumulate
FINAL_CFG = dict(
    tile_sizes=TAPER, label_cast="none", sw_pipe=True, eps_ln=True,
    n_gbank=2, sw_depth=2, mid_bufs=3, label_bufs=4,
)


def _get_nc():
    global _NC
    if _NC is None:
        _NC = build_nc(**FINAL_CFG)
    return _NC


def kernel(pred_all: np.ndarray, label: np.ndarray) -> np.ndarray:
    assert pred_all.shape == (B, NCOL) and label.shape == (B, NSEL)
    nc = _get_nc()
    pred_all = np.ascontiguousarray(pred_all, dtype=np.float32)
    label = np.ascontiguousarray(label, dtype=np.int32)
    in_maps = [
        {
            "pred": pred_all[c * B_SHARD : (c + 1) * B_SHARD],
            "label": label[c * B_SHARD : (c + 1) * B_SHARD],
        }
        for c in range(N_CORES)
    ]
    r = run_bass_kernel_spmd(nc, in_maps, list(range(N_CORES)))
    total = 0.0
    for c in range(N_CORES):
        total += g_to_partial(r.results[c]["g_out"])
    return np.float32(total)


def g_to_partial(g: np.ndarray) -> float:
    """Sum of diagonal 18x18 blocks of each GDIM-wide slab of G, scaled by
    -1/18. When G carries an extra column of f column-sums (relu_clamp),
    remove the +100 shift."""
    s = 0.0
    if g.shape[1] == GDIM + 1:
        for b_ in range(U):
            s += float(
                g[b_ * NSEL : (b_ + 1) * NSEL, b_ * NSEL : (b_ + 1) * NSEL].sum()
            )
        s -= 100.0 * NSEL * float(g[:, GDIM].sum())
        return -s / NSEL
    for g0 in range(0, g.shape[1], GDIM):
        for b_ in range(U):
            s += float(
                g[b_ * NSEL : (b_ + 1) * NSEL,
                  g0 + b_ * NSEL : g0 + (b_ + 1) * NSEL].sum()
            )
    return -s / NSEL


if __name__ == "__main__":
    rng = np.random.default_rng(0)
    p = rng.random((B, NCOL), dtype=np.float32)
    t = rng.integers(0, 2, size=(B, NSEL)).astype(np.int32)
    print(kernel(p, t))



# revision 50
# speedup vs baseline: 1.0802x; 1.0802x over previous
"""AttrPredLoss_40 focal-BCE loss kernel for Trainium2 (8 NeuronCores, data parallel).

Math (per sample row, 18 selected attrs j):
    p   = pred[:, SEL]                      # in (0,1)
    t   = label in {0,1}
    d   = t - p
    # BCE log arg: t==1 -> p, t==0 -> 1-p  ==  1-|d|
    l   = max(ln(1-|d|), -100)              # sum_j l = -18*bce_mean
    # focal weight: (t?0.8:0.2) * (t?1-p:p)^2 == (t+1/3)*0.6*d^2
    f   = (t + 1/3) * 0.6 * d^2
    loss = sum_rows (sum_j f) * (-1/18) * (sum_j l)

The double reduction + row product is computed on the TensorEngine:
    total = sum_r (sum_j f_rj)(sum_k l_rk) = sum over diagonal 18x18 blocks of
    G += f_chunk^T @ l_chunk  accumulated in PSUM over all 128-row groups,
    chunked 4 row-groups (72 cols) per matmul.
"""

import math
from contextlib import ExitStack

import numpy as np

import concourse.bacc as bacc
import concourse.bass as bass
import concourse.mybir as mybir
import concourse.tile as tile
from concourse.bass_utils import run_bass_kernel_spmd

F32 = mybir.dt.float32
I32 = mybir.dt.int32
BF16 = mybir.dt.bfloat16
F16 = mybir.dt.float16
ALU = mybir.AluOpType
ACTF = mybir.ActivationFunctionType

# selected attribute indices, as contiguous runs: (label_col, pred_col, len)
RUNS = [
    (0, 4, 1),
    (1, 8, 2),
    (3, 11, 1),
    (4, 13, 6),
    (10, 20, 1),
    (11, 22, 2),
    (13, 26, 1),
    (14, 29, 3),
    (17, 36, 1),
]
# same 18 columns as 6 ops: (label_col0, pred_col0, nrun, lstep, pstep, len)
# pairs of runs whose spacing is arithmetic on BOTH the label and pred side
MERGED_RUNS = [
    (0, 4, 1, 0, 0, 1),     # {4}
    (1, 8, 2, 10, 14, 2),   # {8,9} + {22,23}
    (3, 11, 2, 10, 15, 1),  # {11} + {26}
    (4, 13, 1, 0, 0, 6),    # {13..18}
    (10, 20, 2, 7, 16, 1),  # {20} + {36}
    (14, 29, 1, 0, 0, 3),   # {29,30,31}
]
NSEL = 18
NCOL = 40

N_CORES = 8
B = 1_048_576
B_SHARD = B // N_CORES  # 131072
P = 128  # SBUF partitions
U = 4  # row-groups per matmul chunk (must divide r; 18*4 = 72 partitions)
GDIM = NSEL * U  # 72


def build_nc(
    b_shard: int = B_SHARD,
    r: int = 128,
    label_cast: str = "dma",
    loop_n: int = 1,
    io_bufs: int = 2,
    variant: str = "full",
    tile_sizes: list | None = None,
    label_bufs: int | None = None,
    clamp_engine: str = "dve",
    loop_staggered: bool = False,
    sw_pipe: bool = False,
    relu_clamp: bool = False,
    eps_ln: bool = False,
    act_order: str = "sq_first",
    mid_bufs: int = 2,
    abs_engine: str = "act",
    mid16: bool = False,
    pred16: bool = False,
    sub_engine: str = "dve",
    pack: bool = False,
    d16: bool = False,
    n_gbank: int = 1,
    label_queue: str = "sync",
    passes: int = 1,
    back_first: bool = False,
    dma_span: int = 1,
    s6_psum: bool = False,
    big_mode: bool = False,
    eps_imm: bool = False,
    sw_depth: int = 1,
    da_bufs: int = 0,
):
    """Build the per-core Bass module. b_shard rows per core; r rows per
    partition per tile (or explicit tile_sizes list summing to nrows).
    loop_n>1 wraps the body in a device-side For loop (same data re-read
    each iteration) for wall-clock benchmarking."""
    assert b_shard % P == 0
    nrows = b_shard // P
    if tile_sizes is None:
        assert nrows % r == 0
        tile_sizes = [r] * (nrows // r)
    assert sum(tile_sizes) == nrows, (tile_sizes, nrows)
    assert all(t % U == 0 for t in tile_sizes)

    nc = bacc.Bacc("TRN2", target_bir_lowering=False, debug=False)

    pred = nc.dram_tensor("pred", [b_shard, NCOL], F32, kind="ExternalInput")
    label = nc.dram_tensor("label", [b_shard, NSEL], I32, kind="ExternalInput")
    assert n_gbank == 1 or not relu_clamp
    gw = GDIM + 1 if relu_clamp else n_gbank * GDIM
    gout = nc.dram_tensor("g_out", [GDIM, gw], F32, kind="ExternalOutput")

    # blocked layout: partition p holds rows [p*nrows, (p+1)*nrows)
    pred_r = pred.ap().rearrange("(p n) c -> p n c", p=P)
    label_r = label.ap().rearrange("(p n) c -> p n c", p=P)

    s6scale = math.sqrt(0.6)

    with tile.TileContext(nc) as tc, ExitStack() as ctx:
        io = ctx.enter_context(tc.tile_pool(name="io", bufs=io_bufs))
        psum2 = None
        if s6_psum:
            psum2 = ctx.enter_context(
                tc.tile_pool(name="psum2", bufs=mid_bufs, space="PSUM")
            )
        da = None
        if big_mode or da_bufs:
            # d and a die within their own wave, so they don't need pools
            # scaled to the software-pipeline depth like s6/l/f do
            da = ctx.enter_context(
                tc.tile_pool(name="da", bufs=da_bufs or 2)
            )
        if label_bufs is None:
            iol = io
        else:
            iol = ctx.enter_context(tc.tile_pool(name="iol", bufs=label_bufs))
        mid = ctx.enter_context(tc.tile_pool(name="mid", bufs=mid_bufs))
        singles = ctx.enter_context(tc.tile_pool(name="singles", bufs=1))
        psum = ctx.enter_context(tc.tile_pool(name="psum", bufs=1, space="PSUM"))

        Gb = []
        for b_i in range(n_gbank):
            # pad each accumulator to a full 2KiB PSUM bank: matmul
            # start=True clears has_written bits bank-wide, so two
            # accumulation chains sharing a bank would corrupt each other
            Gb.append(
                psum.tile(
                    [GDIM, GDIM], F32, name=f"Gbank{b_i}", tag=f"G{b_i}",
                    padded_shape=[GDIM, 512],
                )
            )
        G = Gb[0]
        G2 = None
        ones1 = None
        if relu_clamp:
            G2 = psum.tile([GDIM, 1], F32)
            ones1 = singles.tile([P, 1], F16)
            nc.vector.memset(ones1[:], 1.0)
            c100 = singles.tile([P, 1], F32)
            nc.vector.memset(c100[:], 100.0)
        ceps = None
        if eps_ln:
            ceps = singles.tile([P, 1], F32)
            nc.vector.memset(ceps[:], 1.0 + 2.0**-23)

        total_chunks = sum(t // U for t in tile_sizes) * passes
        assert total_chunks % n_gbank == 0
        chunk_idx = 0

        if loop_n > 1:
            loop_cm = tc.For_i(0, loop_n, 1, staggered_reset=loop_staggered)
        else:
            loop_cm = None
        if loop_cm is not None:
            ctx.enter_context(loop_cm)

        def emit_front(rt, rsl, pt=None, tt=None):
            """DMA + ops that depend only on this tile's DMA. When pt/tt are
            given (dma_span>1), the DMA was already issued for a span of
            tiles and pt/tt are sub-views of the span buffer."""
            if pt is not None:
                return emit_compute(rt, pt, tt)
            if pred16:
                # SWDGE cast f32 -> bf16 during the DMA (halves pred SBUF)
                pt = io.tile([P, rt, NCOL], BF16, tag="pred")
                nc.gpsimd.dma_start(out=pt[:], in_=pred_r[:, rsl, :])
            else:
                pt = io.tile([P, rt, NCOL], F32, tag="pred")
                nc.sync.dma_start(out=pt[:], in_=pred_r[:, rsl, :])

            if label_cast == "dma":
                tt = iol.tile([P, rt, NSEL], F32, tag="label")
                nc.gpsimd.dma_start(out=tt[:], in_=label_r[:, rsl, :])
            elif label_cast == "dma16":
                # SWDGE cast i32 -> bf16 during the DMA; 16-bit labels let
                # the f STT run in 2x_1P mode when s6 is also bf16
                tt = iol.tile([P, rt, NSEL], BF16, tag="label")
                nc.gpsimd.dma_start(out=tt[:], in_=label_r[:, rsl, :])
            elif label_cast == "none":
                # no cast: DVE ops read the i32 labels with on-read convert
                tt = iol.tile([P, rt, NSEL], I32, tag="label")
                lq = {"sync": nc.sync, "scalar": nc.scalar,
                      "gpsimd": nc.gpsimd}[label_queue]
                lq.dma_start(out=tt[:], in_=label_r[:, rsl, :])
            else:
                ti = iol.tile([P, rt, NSEL], I32, tag="label_i")
                nc.sync.dma_start(out=ti[:], in_=label_r[:, rsl, :])
                tt = iol.tile([P, rt, NSEL], F32, tag="label")
                nc.gpsimd.tensor_copy(
                    out=tt.rearrange("p n c -> p (n c)"),
                    in_=ti.rearrange("p n c -> p (n c)"),
                )

            if variant == "dma_only":
                return None
            return emit_compute(rt, pt, tt)

        def emit_compute(rt, pt, tt):
            dt_d = BF16 if (pred16 or pack or d16) else F32
            # d = t - p on the 18 selected columns
            if big_mode:
                d = da.tile([P, rt, NSEL], F32, tag="da")
            elif da_bufs:
                d = da.tile([P, rt, NSEL], dt_d, tag="d")
            else:
                d = mid.tile([P, rt, NSEL], dt_d, tag="d")

            def run_view(tile3, col0, nrun, step, ln_):
                base = tile3[:]
                ap = [base.ap[0], base.ap[1]]
                if nrun > 1:
                    ap.append([step, nrun])
                ap.append([1, ln_])
                return bass.AP(tensor=base.tensor, offset=base.offset + col0, ap=ap)

            if pack:
                # gather+cast pred's 18 selected cols into a contiguous bf16
                # tile on the (otherwise idle) GPSIMD engine, then one
                # contiguous DVE sub in 2x mode
                pp = mid.tile([P, rt, NSEL], BF16, tag="pp")
                for lc0, pc0, nrun, lstep, pstep, ln_ in MERGED_RUNS:
                    nc.gpsimd.tensor_copy(
                        out=run_view(pp, lc0, nrun, lstep, ln_),
                        in_=run_view(pt, pc0, nrun, pstep, ln_),
                    )
                nc.vector.tensor_sub(
                    d.rearrange("p n c -> p (n c)"),
                    tt.rearrange("p n c -> p (n c)"),
                    pp.rearrange("p n c -> p (n c)"),
                )
            else:
                # 6 merged-run sub ops, optionally offloading the biggest
                # runs to GPSIMD
                for i, (lc0, pc0, nrun, lstep, pstep, ln_) in enumerate(MERGED_RUNS):
                    eng = nc.vector
                    if sub_engine == "gp6" or (
                        sub_engine == "gp3" and ln_ >= 2
                    ) or (sub_engine == "gp1" and ln_ == 6):
                        eng = nc.gpsimd
                    eng.tensor_sub(
                        run_view(d, lc0, nrun, lstep, ln_),
                        run_view(tt, lc0, nrun, lstep, ln_),
                        run_view(pt, pc0, nrun, pstep, ln_),
                    )

            if variant == "dpass":
                return None

            dflat = d.rearrange("p n c -> p (n c)")

            if s6_psum:
                # ScalarE writes PSUM faster than SBUF, and this takes the
                # s6 traffic off the SBUF ports entirely (bf16 to fit 2 bufs)
                s6 = psum2.tile([P, rt * NSEL], BF16, tag="s6")
            else:
                s6 = mid.tile([P, rt * NSEL], BF16 if mid16 else F32, tag="s6")
            if big_mode:
                # a = |d| via sign-clear, then BOTH Square and Ln read a
                # (d^2 == |d|^2), so d's buffer frees right after the AND
                # and d/a can share one double-buffered pool
                a3 = da.tile([P, rt, NSEL], F32, tag="da")
                a = a3.rearrange("p n c -> p (n c)")
                nc.vector.tensor_scalar(
                    a.bitcast(I32), dflat.bitcast(I32), 0x7FFFFFFF,
                    None, ALU.bitwise_and,
                )
                nc.scalar.activation(s6[:], a, ACTF.Square, scale=s6scale)
            elif da_bufs:
                a3 = da.tile([P, rt, NSEL], dt_d, tag="a")
                a = a3.rearrange("p n c -> p (n c)")
            else:
                a = mid.tile([P, rt * NSEL], dt_d, tag="a")
            if big_mode:
                pass
            elif abs_engine == "dve_and":
                # a = |d| on the DVE: clear the sign bit via a single-src
                # tensor_scalar bitwise_and on an int bitcast view (2x/4x
                # mode). Takes Abs off the ScalarE chain (ACT: Square + Ln).
                if dt_d == BF16:
                    nc.vector.tensor_scalar(
                        a[:].bitcast(mybir.dt.int16),
                        dflat.bitcast(mybir.dt.int16), 0x7FFF, None,
                        ALU.bitwise_and,
                    )
                else:
                    nc.vector.tensor_scalar(
                        a[:].bitcast(I32), dflat.bitcast(I32), 0x7FFFFFFF,
                        None, ALU.bitwise_and,
                    )
                if act_order == "sq_first":
                    nc.scalar.activation(s6[:], dflat, ACTF.Square, scale=s6scale)
            elif abs_engine == "dve_ttabs":
                # a = |d| via tensor_tensor abs_max(d, d) (1x fp32)
                nc.vector.tensor_tensor(a[:], dflat, dflat, ALU.abs_max)
                if act_order == "sq_first":
                    nc.scalar.activation(s6[:], dflat, ACTF.Square, scale=s6scale)
            elif act_order == "sq_first":
                # s6 = 0.6 * d^2   (ACT, scale applied before Square)
                nc.scalar.activation(s6[:], dflat, ACTF.Square, scale=s6scale)
                # a = |d|   (ScalarE Abs)
                nc.scalar.activation(a[:], dflat, ACTF.Abs)
            else:
                # Abs/Ln first: lc reaches the PE sooner; Square's consumer
                # (the f STT) runs one wave later anyway under sw_pipe
                nc.scalar.activation(a[:], dflat, ACTF.Abs)

            if eps_ln:
                # l = Ln(1 + 2^-23 - |d|): the epsilon keeps the argument
                # positive (p=0 -> l ~ -15.9 instead of -inf), making the
                # -100 clamp unnecessary (error impact ~3e-6 of the total).
                # Write bf16 directly for the matmul.
                l = mid.tile([P, rt * NSEL], BF16, tag="l")
                nc.scalar.activation(
                    l[:], a[:], ACTF.Ln,
                    bias=(1.0 + 2.0**-23) if eps_imm else ceps[:, 0:1],
                    scale=-1.0,
                )
            else:
                # l = Ln(1 - |d|)
                l = mid.tile([P, rt * NSEL], F32, tag="l")
                nc.scalar.activation(l[:], a[:], ACTF.Ln, bias=1.0, scale=-1.0)

            if act_order != "sq_first":
                nc.scalar.activation(s6[:], dflat, ACTF.Square, scale=s6scale)

            if variant == "acts":
                return None

            return (rt, tt, s6, l)

        def emit_back(state):
            """DVE/PE ops that consume ACT results (run one tile later when
            sw_pipe so the in-order DVE queue never waits on same-tile ACT)."""
            nonlocal chunk_idx
            rt, tt, s6, l = state
            ttflat = tt.rearrange("p n c -> p (n c)")

            # f = (t + 1/3) * s6 = (0.2 + 0.6t) * d^2  (dtype matches lc)
            f = mid.tile([P, rt * NSEL], F16 if relu_clamp else BF16, tag="f")
            nc.vector.scalar_tensor_tensor(
                f[:], ttflat, 1.0 / 3.0, s6[:], ALU.add, ALU.mult
            )

            if eps_ln:
                lc = l  # already clamped-by-construction, bf16
            elif relu_clamp:
                # lc' = relu(l + 100) = max(l,-100) + 100 on ScalarE (fp16);
                # the +100 is removed exactly via G2 = column sums of f
                lc = mid.tile([P, rt * NSEL], F16, tag="lc")
                nc.scalar.activation(lc[:], l[:], ACTF.Relu, bias=c100[:, 0:1], scale=1.0)
            else:
                # lc = max(l, -100)  -> bf16 for the matmul
                lc = mid.tile([P, rt * NSEL], BF16, tag="lc")
                if clamp_engine == "gpsimd":
                    nc.gpsimd.tensor_scalar_max(lc[:], l[:], -100.0)
                else:
                    nc.vector.tensor_scalar(lc[:], l[:], -100.0, None, ALU.max)

            if variant == "no_pe":
                return

            # G += f_chunk^T @ lc_chunk over chunks of U row-groups;
            # n_gbank>1 ping-pongs PSUM accumulators so consecutive matmuls
            # don't serialize on the same-PSUM-address accumulate
            done = 0
            while done < rt:
                u = min(U, rt - done)
                m = NSEL * u
                sl = slice(done * NSEL, (done + u) * NSEL)
                nc.tensor.matmul(
                    out=Gb[chunk_idx % n_gbank][0:m, 0:m],
                    lhsT=f[:, sl],
                    rhs=lc[:, sl],
                    start=(chunk_idx < n_gbank),
                    stop=(chunk_idx >= total_chunks - n_gbank),
                )
                if relu_clamp:
                    nc.tensor.matmul(
                        out=G2[0:m, 0:1],
                        lhsT=f[:, sl],
                        rhs=ones1[:, 0:1],
                        start=(chunk_idx == 0),
                        stop=(chunk_idx == total_chunks - 1),
                    )
                chunk_idx += 1
                done += u

        if dma_span > 1:
            assert label_cast == "none" and not pred16

        pendings = []
        for _pass in range(passes):
            row0 = 0
            idx = 0
            while idx < len(tile_sizes):
                span = 1
                while (
                    span < dma_span
                    and idx + span < len(tile_sizes)
                    and tile_sizes[idx + span] == tile_sizes[idx]
                ):
                    span += 1
                group = tile_sizes[idx : idx + span]
                idx += span
                gt = sum(group)
                ptb = ttb = None
                if span > 1:
                    # one DMA covering the whole span (fewer, larger DMAs)
                    ptb = io.tile([P, gt, NCOL], F32, tag="pred")
                    nc.sync.dma_start(
                        out=ptb[:], in_=pred_r[:, row0 : row0 + gt, :]
                    )
                    ttb = iol.tile([P, gt, NSEL], I32, tag="label")
                    lq = {"sync": nc.sync, "scalar": nc.scalar,
                          "gpsimd": nc.gpsimd}[label_queue]
                    lq.dma_start(
                        out=ttb[:], in_=label_r[:, row0 : row0 + gt, :]
                    )
                for k, rt in enumerate(group):
                    rsl = slice(row0, row0 + rt)
                    row0 += rt
                    if sw_pipe and back_first and pendings:
                        # emit the previous tile's DVE/PE consumers BEFORE
                        # this tile's front so the in-order DVE queue can run
                        # them while this tile's DMA is still in flight
                        for st_ in pendings:
                            emit_back(st_)
                        pendings = []
                    if span > 1:
                        if variant == "dma_only":
                            st = None
                        else:
                            st = emit_front(
                                rt, rsl,
                                pt=ptb[:, k * rt : (k + 1) * rt, :],
                                tt=ttb[:, k * rt : (k + 1) * rt, :],
                            )
                    else:
                        st = emit_front(rt, rsl)
                    if st is None:
                        continue
                    if not sw_pipe:
                        emit_back(st)
                    else:
                        pendings.append(st)
                        while len(pendings) > sw_depth:
                            emit_back(pendings.pop(0))
        for st_ in pendings:
            emit_back(st_)

        # epilogue: ship G (and the f column sums) to the host
        Gs = singles.tile([GDIM, gw], F32)
        if variant == "full":
            for b_i in range(n_gbank):
                nc.scalar.copy(Gs[:, b_i * GDIM : (b_i + 1) * GDIM], Gb[b_i][:])
            if relu_clamp:
                nc.scalar.copy(Gs[:, GDIM : GDIM + 1], G2[:])
        else:
            nc.vector.memset(Gs[:], 0.0)
        nc.sync.dma_start(out=gout.ap(), in_=Gs[:])

    nc.compile()
    return nc


_NC = None


TAPER = [128] * 7 + [64, 48, 16]  # smaller final tiles shorten the compute tail

# HWDGE label loads (mixed-dtype i32 reads on DVE) + TWO-tile software
# pipeline (consumers run two waves late, fully decoupling the STT/matmul
# back-phase from the ACT chain; needs deeper mid/label pools) +
# epsilon-biased Ln that makes the -100 clamp unnecessary + two
# ping-ponged PSUM accumulators so consecutive matmuls don't serialize
# on the same-bank accumulate
FINAL_CFG = dict(
    tile_sizes=TAPER, label_cast="none", sw_pipe=True, eps_ln=True,
    n_gbank=2, sw_depth=2, mid_bufs=3, label_bufs=4,
)


def _get_nc():
    global _NC
    if _NC is None:
        _NC = build_nc(**FINAL_CFG)
    return _NC


def kernel(pred_all: np.ndarray, label: np.ndarray) -> np.ndarray:
    assert pred_all.shape == (B, NCOL) and label.shape == (B, NSEL)
    nc = _get_nc()
    pred_all = np.ascontiguousarray(pred_all, dtype=np.float32)
    label = np.ascontiguousarray(label, dtype=np.int32)
    in_maps = [
        {
            "pred": pred_all[c * B_SHARD : (c + 1) * B_SHARD],
            "label": label[c * B_SHARD : (c + 1) * B_SHARD],
        }
        for c in range(N_CORES)
    ]
    r = run_bass_kernel_spmd(nc, in_maps, list(range(N_CORES)))
    total = 0.0
    for c in range(N_CORES):
        total += g_to_partial(r.results[c]["g_out"])
    return np.float32(total)


def g_to_partial(g: np.ndarray) -> float:
    """Sum of diagonal 18x18 blocks of each GDIM-wide slab of G, scaled by
    -1/18. When G carries an extra column of f column-sums (relu_clamp),
    remove the +100 shift."""
    s = 0.0
    if g.shape[1] == GDIM + 1:
        for b_ in range(U):
            s += float(
                g[b_ * NSEL : (b_ + 1) * NSEL, b_ * NSEL : (b_ + 1) * NSEL].sum()
            )
        s -= 100.0 * NSEL * float(g[:, GDIM].sum())
        return -s / NSEL
    for g0 in range(0, g.shape[1], GDIM):
        for b_ in range(U):
            s += float(
                g[b_ * NSEL : (b_ + 1) * NSEL,
                  g0 + b_ * NSEL : g0 + (b_ + 1) * NSEL].sum()
            )
    return -s / NSEL


if __name__ == "__main__":
    rng = np.random.default_rng(0)
    p = rng.random((B, NCOL), dtype=np.float32)
    t = rng.integers(0, 2, size=(B, NSEL)).astype(np.int32)
    print(kernel(p, t))

